# revision 3
# baseline (speedup 1.0000x reference)
"""CLD sde_reverse (Riemann geometry) Trainium2 kernel.

Contract: kernel(u, score_x, t) -> (drift, diffusion), full (unsharded) numpy
arrays, computed on 8 NeuronCores via bass/Tile + run_bass_kernel_spmd.

Sharding: pixels (image rows) are sharded 8 ways; every core sees all 64 batch
elements for its 32 rows. The batch-mean outer product G, the 3x3
inverse/cholesky, and the drift matmuls are all per-pixel, so there are no
cross-core dependencies and no collectives.

Math (per pixel, 3x3):
    G     = alpha * (mean_b s s^T)/norm + (1-alpha)/m_inv * I
    L     = chol(G),  Ginv = adj(G)/det(G)
    A     = beta * L @ Ginv
    drift_x = A @ r
    drift_r = -(beta*L) @ x - beta*Gamma * G @ (Ginv @ r)
            = -(beta*L) @ x - beta*Gamma * r          (G @ Ginv = I exactly)
    diffusion_x = 0
    diffusion_r = sqrt(2*beta*Gamma) * (L @ 1)        (batch independent)

Device layout per core: pixel p in [0,8192) maps to (part, pl) = (p>>6, p&63);
tensors are [channel, 128 part, 64 batch, 64 pl] so every DMA run is
contiguous.  G & coefficients are fp32; the big batched elementwise stage runs
in fp16 (rel err ~1e-3 vs the fp32 reference).
"""

import math

import numpy as np

# ---- model constants (from the reference config) ----
M_INV = 4.0
GAMMA_BIG = 0.04
BETA0 = 4.0
RIEMANN_MIX = 0.5
K_DECAY = 4.5
C = 3
HW = 256
B = 64

N_CORES = 8
ROWS = HW // N_CORES  # 32 rows per core
P = 128               # SBUF partitions
PL = (ROWS * HW) // P  # 64 free pixels per partition

BETA_C = BETA0 * math.sqrt(M_INV)        # 8.0
GAMMA_C = GAMMA_BIG * math.sqrt(M_INV)   # 0.08
BG = BETA_C * GAMMA_C                    # 0.64
BG_SCALE = math.sqrt(2.0 * BETA_C * GAMMA_C)

_PROG_CACHE: dict = {}


def _build_program(ca: float, cid: float, main_fp16: bool = True):
    """Build + compile the per-core SPMD bass program.

    ca  = alpha / (B * normalization)   (scale for the raw sum S_ij)
    cid = (1 - alpha) / M_INV           (identity mixture term)
    """
    import concourse.bacc as bacc
    import concourse.mybir as mybir
    import concourse.tile as tile

    dt = mybir.dt
    op = mybir.AluOpType
    f32 = dt.float32
    f16 = dt.float16 if main_fp16 else dt.float32
    AF = mybir.ActivationFunctionType

    nc = bacc.Bacc("TRN2", target_bir_lowering=False, debug=False,
                   num_devices=N_CORES)

    s_in = nc.dram_tensor("s_in", [C, P, B, PL], f32, kind="ExternalInput").ap()
    u_in = nc.dram_tensor("u_in", [2 * C, P, B, PL], f16,
                          kind="ExternalInput").ap()
    drift_o = nc.dram_tensor("drift", [2 * C, P, B, PL], f16,
                             kind="ExternalOutput").ap()
    dif_o = nc.dram_tensor("dif", [C, P, PL], f32, kind="ExternalOutput").ap()

    PAIRS = [(0, 0), (0, 1), (0, 2), (1, 1), (1, 2), (2, 2)]

    with tile.TileContext(nc) as tc:
        with (
            tc.tile_pool(name="coef", bufs=1) as coef,   # [P, PL] fp32 planes
            tc.tile_pool(name="data", bufs=1) as data,   # fp16 channel tiles
            tc.tile_pool(name="tmp", bufs=2) as tmp,
        ):
            # ---------------- stage A: G_ij = ca * sum_b s_i s_j + cid*I ----
            g = {}
            with tc.tile_pool(name="score", bufs=1) as score_pool, \
                 tc.tile_pool(name="prod", bufs=2) as prod_pool:
                s_t = []
                for c in range(C):
                    st = score_pool.tile([P, B, PL], f32, tag=f"s{c}")
                    nc.sync.dma_start(out=st[:], in_=s_in[c])
                    s_t.append(st)
                # pre-load u (overlaps with compute below)
                u_t = []
                for c in range(2 * C):
                    ut = data.tile([P, B, PL], f16, tag=f"u{c}")
                    nc.sync.dma_start(out=ut[:], in_=u_in[c])
                    u_t.append(ut)
                x_t, r_t = u_t[:C], u_t[C:]

                for (i, j) in PAIRS:
                    pt = prod_pool.tile([P, B, PL], f32, tag="prod")
                    nc.vector.tensor_tensor(pt[:], s_t[i][:], s_t[j][:],
                                            op.mult)
                    sij = tmp.tile([P, PL], f32, tag=f"S{i}{j}")
                    nc.vector.tensor_reduce(
                        sij[:], pt[:].rearrange("p b l -> p l b"),
                        axis=mybir.AxisListType.X, op=op.add)
                    gij = coef.tile([P, PL], f32, tag=f"g{i}{j}")
                    nc.vector.tensor_scalar(
                        gij[:], sij[:], float(ca),
                        float(cid) if i == j else 0.0, op.mult, op.add)
                    g[(i, j)] = gij
                    g[(j, i)] = gij

            # ---------------- stage B: per-pixel 3x3 coefficients ----------
            def tt(a, b_, o, tag):
                t = coef.tile([P, PL], f32, tag=tag)
                nc.vector.tensor_tensor(t[:], a[:], b_[:], o)
                return t

            def fmsub(a, b_, c_, d_, tag):
                # a*b - c*d
                t1 = tmp.tile([P, PL], f32, tag="fm1")
                nc.vector.tensor_tensor(t1[:], a[:], b_[:], op.mult)
                t2 = tmp.tile([P, PL], f32, tag="fm2")
                nc.vector.tensor_tensor(t2[:], c_[:], d_[:], op.mult)
                t = coef.tile([P, PL], f32, tag=tag)
                nc.vector.tensor_tensor(t[:], t1[:], t2[:], op.subtract)
                return t

            # adjugate (symmetric): c00 = g11*g22 - g12^2, ...
            c00 = fmsub(g[1, 1], g[2, 2], g[1, 2], g[1, 2], "c00")
            c01 = fmsub(g[0, 2], g[1, 2], g[0, 1], g[2, 2], "c01")
            c02 = fmsub(g[0, 1], g[1, 2], g[0, 2], g[1, 1], "c02")
            c11 = fmsub(g[0, 0], g[2, 2], g[0, 2], g[0, 2], "c11")
            c12 = fmsub(g[0, 1], g[0, 2], g[0, 0], g[1, 2], "c12")
            c22 = fmsub(g[0, 0], g[1, 1], g[0, 1], g[0, 1], "c22")

            # det = g00*c00 + g01*c01 + g02*c02
            d0 = tt(g[0, 0], c00, op.mult, "d0")
            d1 = tt(g[0, 1], c01, op.mult, "d1")
            d2 = tt(g[0, 2], c02, op.mult, "d2")
            det = tt(d0, d1, op.add, "deta")
            det = tt(det, d2, op.add, "det")
            rdet = coef.tile([P, PL], f32, tag="rdet")
            nc.vector.reciprocal(rdet[:], det[:])

            iv = {}
            for (i, j), cof in [((0, 0), c00), ((0, 1), c01), ((0, 2), c02),
                                ((1, 1), c11), ((1, 2), c12), ((2, 2), c22)]:
                ivt = tt(cof, rdet, op.mult, f"iv{i}{j}")
                iv[(i, j)] = ivt
                iv[(j, i)] = ivt

            # cholesky (one Newton step on the ACT sqrt)
            def sqrt_ref(a, tag):
                s0 = tmp.tile([P, PL], f32, tag="sq0")
                nc.scalar.activation(s0[:], a[:], AF.Sqrt)
                r0 = tmp.tile([P, PL], f32, tag="sqr")
                nc.vector.reciprocal(r0[:], s0[:])
                ar = tmp.tile([P, PL], f32, tag="sqar")
                nc.vector.tensor_tensor(ar[:], a[:], r0[:], op.mult)
                ssum = tmp.tile([P, PL], f32, tag="sqsum")
                nc.vector.tensor_tensor(ssum[:], s0[:], ar[:], op.add)
                out = coef.tile([P, PL], f32, tag=tag)
                nc.vector.tensor_scalar(out[:], ssum[:], 0.5, None, op.mult)
                return out

            l00 = sqrt_ref(g[0, 0], "l00")
            il00 = coef.tile([P, PL], f32, tag="il00")
            nc.vector.reciprocal(il00[:], l00[:])
            l10 = tt(g[0, 1], il00, op.mult, "l10")
            l20 = tt(g[0, 2], il00, op.mult, "l20")
            t = tt(l10, l10, op.mult, "l10sq")
            dd1 = tt(g[1, 1], t, op.subtract, "dd1")
            l11 = sqrt_ref(dd1, "l11")
            il11 = coef.tile([P, PL], f32, tag="il11")
            nc.vector.reciprocal(il11[:], l11[:])
            t = tt(l20, l10, op.mult, "l20l10")
            t = tt(g[1, 2], t, op.subtract, "g12m")
            l21 = tt(t, il11, op.mult, "l21")
            t = tt(l20, l20, op.mult, "l20sq")
            dd2 = tt(g[2, 2], t, op.subtract, "dd2a")
            t = tt(l21, l21, op.mult, "l21sq")
            dd2 = tt(dd2, t, op.subtract, "dd2")
            l22 = sqrt_ref(dd2, "l22")

            # bL = beta * L
            L = {}
            for (i, j), lt in [((0, 0), l00), ((1, 0), l10), ((1, 1), l11),
                               ((2, 0), l20), ((2, 1), l21), ((2, 2), l22)]:
                blt = coef.tile([P, PL], f32, tag=f"bl{i}{j}")
                nc.vector.tensor_scalar(blt[:], lt[:], BETA_C, None, op.mult)
                L[(i, j)] = blt

            # A = bL @ Ginv  (A_ij = sum_{k<=i} bL_ik * Ginv_kj)
            A = {}
            for jj in range(3):
                A[(0, jj)] = tt(L[0, 0], iv[0, jj], op.mult, f"A0{jj}")
                t1 = tmp.tile([P, PL], f32, tag="Am1")
                nc.vector.tensor_tensor(t1[:], L[1, 0][:], iv[0, jj][:],
                                        op.mult)
                t2 = tmp.tile([P, PL], f32, tag="Am2")
                nc.vector.tensor_tensor(t2[:], L[1, 1][:], iv[1, jj][:],
                                        op.mult)
                A[(1, jj)] = tt(t1, t2, op.add, f"A1{jj}")
                t1 = tmp.tile([P, PL], f32, tag="Am3")
                nc.vector.tensor_tensor(t1[:], L[2, 0][:], iv[0, jj][:],
                                        op.mult)
                t2 = tmp.tile([P, PL], f32, tag="Am4")
                nc.vector.tensor_tensor(t2[:], L[2, 1][:], iv[1, jj][:],
                                        op.mult)
                t1b = tmp.tile([P, PL], f32, tag="Am5")
                nc.vector.tensor_tensor(t1b[:], t1[:], t2[:], op.add)
                t2b = tmp.tile([P, PL], f32, tag="Am6")
                nc.vector.tensor_tensor(t2b[:], L[2, 2][:], iv[2, jj][:],
                                        op.mult)
                A[(2, jj)] = tt(t1b, t2b, op.add, f"A2{jj}")

            # diffusion_r rows: bg_scale/beta * sum_j bL_ij -> dram
            bgob = BG_SCALE / BETA_C
            dif0 = coef.tile([P, PL], f32, tag="dif0")
            nc.vector.tensor_scalar(dif0[:], L[0, 0][:], bgob, None, op.mult)
            t = tt(L[1, 0], L[1, 1], op.add, "difs1")
            dif1 = coef.tile([P, PL], f32, tag="dif1")
            nc.vector.tensor_scalar(dif1[:], t[:], bgob, None, op.mult)
            t = tt(L[2, 0], L[2, 1], op.add, "difs2a")
            t = tt(t, L[2, 2], op.add, "difs2")
            dif2 = coef.tile([P, PL], f32, tag="dif2")
            nc.vector.tensor_scalar(dif2[:], t[:], bgob, None, op.mult)
            for i, dtile in enumerate((dif0, dif1, dif2)):
                nc.sync.dma_start(out=dif_o[i], in_=dtile[:])

            # ---------------- stage C: batched elementwise main stage ------
            # Coefficients are read through broadcast APs ([P,1,PL] fp16
            # tiles, batch dim broadcast with step 0) — no expansion pass.
            def to16(plane, tag):
                e = coef.tile([P, 1, PL], f16, tag=tag)
                nc.vector.tensor_copy(e[:, 0, :], plane[:])
                return e[:].broadcast_to([P, B, PL])

            eA = {(i, j): to16(A[(i, j)], f"eA{i}{j}")
                  for i in range(3) for j in range(3)}
            eL = {(i, j): to16(L[(i, j)], f"eL{i}{j}")
                  for (i, j) in [(0, 0), (1, 0), (1, 1),
                                 (2, 0), (2, 1), (2, 2)]}

            def madd_chain(coeffs, ins, out_tile):
                """out = sum_i coeffs[i] * ins[i]; coeffs are broadcast APs."""
                acc = None
                for idx, (cc, dd) in enumerate(zip(coeffs, ins)):
                    last = idx == len(coeffs) - 1
                    if acc is None:
                        dst = out_tile if last else mtmp.tile(
                            [P, B, PL], f16, tag="mc_acc")
                        nc.vector.tensor_tensor(dst[:], dd[:], cc, op.mult)
                        acc = dst
                    else:
                        pr = mtmp.tile([P, B, PL], f16, tag="mc_pr")
                        nc.vector.tensor_tensor(pr[:], dd[:], cc, op.mult)
                        dst = out_tile if last else mtmp.tile(
                            [P, B, PL], f16, tag="mc_acc")
                        nc.vector.tensor_tensor(dst[:], acc[:], pr[:], op.add)
                        acc = dst
                return acc

            with tc.tile_pool(name="mtmp", bufs=2) as mtmp, \
                 tc.tile_pool(name="outs", bufs=1) as outs:
                # drift_x_i = sum_j A_ij r_j
                for i in range(3):
                    dx = outs.tile([P, B, PL], f16, tag=f"dx{i}")
                    madd_chain([eA[(i, 0)], eA[(i, 1)], eA[(i, 2)]], r_t, dx)
                    nc.sync.dma_start(out=drift_o[i], in_=dx[:])
                # drift_r_i = -(bL x)_i - BG * r_i
                for i in range(3):
                    m = mtmp.tile([P, B, PL], f16, tag="m_i")
                    madd_chain([eL[(i, j)] for j in range(i + 1)],
                               x_t[:i + 1], m)
                    dr = outs.tile([P, B, PL], f16, tag=f"dr{i}")
                    nc.vector.scalar_tensor_tensor(
                        dr[:], r_t[i][:], -BG, m[:], op.mult, op.subtract)
                    nc.sync.dma_start(out=drift_o[C + i], in_=dr[:])

    nc.compile()
    return nc


def _host_reference(u, score_x, t):
    """Pure-numpy fallback (exact reference math); used only when t[0]==1.0
    (the stateful normalization branch, never hit with uniform t)."""
    x, r = u[:, :C], u[:, C:]
    s = np.transpose(score_x, (0, 2, 3, 1)).astype(np.float32)
    G = np.einsum("bhwi,bhwj->hwij", s, s) / np.float32(score_x.shape[0])
    t0 = t[0]
    diag_mean = np.mean(np.trace(G, axis1=-2, axis2=-1)) / C
    normalization = np.where(t0 == 1.0, diag_mean * M_INV, 1.0)
    G = G / normalization
    G_id = (1.0 / M_INV) * np.eye(C, dtype=G.dtype)
    alpha = RIEMANN_MIX * np.exp(-K_DECAY * (1.0 - t0))
    G = alpha * G + (1.0 - alpha) * G_id
    G_inv = np.linalg.inv(G).astype(np.float32)
    G_sqrt = np.linalg.cholesky(G).astype(np.float32)

    def mm(Amat, Bf):
        return np.einsum("hwij,bjhw->bihw", Amat, Bf).astype(np.float32)

    hr = mm(G_inv, r)
    drift_x = BETA_C * mm(G_sqrt, hr)
    drift_r = (-BETA_C * mm(G_sqrt, x) - BETA_C * GAMMA_C * mm(G, hr))
    diffusion_x = np.zeros_like(x)
    diffusion_r = BG_SCALE * mm(G_sqrt, np.ones_like(r))
    drift = np.concatenate((drift_x, drift_r), axis=1)
    diffusion = np.concatenate((diffusion_x, diffusion_r), axis=1)
    return drift, diffusion


def kernel(u, score_x, t):
    from concourse.bass_utils import run_bass_kernel_spmd

    u = np.ascontiguousarray(np.asarray(u, dtype=np.float32))
    score_x = np.ascontiguousarray(np.asarray(score_x, dtype=np.float32))
    t = np.asarray(t, dtype=np.float32)

    t0 = float(t[0])
    if t0 == 1.0:
        return _host_reference(u, score_x, t)

    alpha = RIEMANN_MIX * math.exp(-K_DECAY * (1.0 - t0))
    ca = alpha / B          # normalization == 1.0 on this branch
    cid = (1.0 - alpha) / M_INV

    key = (round(ca, 12), round(cid, 12))
    nc = _PROG_CACHE.get(key)
    if nc is None:
        nc = _build_program(ca, cid)
        _PROG_CACHE[key] = nc

    in_maps = []
    for k in range(N_CORES):
        rows = slice(k * ROWS, (k + 1) * ROWS)
        s_np = (score_x[:, :, rows, :]
                .reshape(B, C, P, PL).transpose(1, 2, 0, 3))
        u_np = (u[:, :, rows, :]
                .reshape(B, 2 * C, P, PL).transpose(1, 2, 0, 3)
                .astype(np.float16))
        in_maps.append({
            "s_in": np.ascontiguousarray(s_np),
            "u_in": np.ascontiguousarray(u_np),
        })

    res = run_bass_kernel_spmd(nc, in_maps, list(range(N_CORES)))

    drift = np.empty((B, 2 * C, HW, HW), dtype=np.float32)
    diffusion = np.zeros((B, 2 * C, HW, HW), dtype=np.float32)
    for k in range(N_CORES):
        rows = slice(k * ROWS, (k + 1) * ROWS)
        dk = res.results[k]["drift"].astype(np.float32)     # [6, P, B, PL]
        drift[:, :, rows, :] = dk.transpose(2, 0, 1, 3).reshape(
            B, 2 * C, ROWS, HW)
        difk = res.results[k]["dif"].reshape(C, ROWS, HW)   # [3, P, PL]
        diffusion[:, C:, rows, :] = difk[None, :, :, :]
    return drift, diffusion
